# revision 10
# baseline (speedup 1.0000x reference)
"""CRF loss (nn_CRFLoss) on 8 Trainium2 NeuronCores.

Strategy
--------
The reference computes, per proposition (B*V = 256 of them):
  logZ via a 128-step forward algorithm over T=66 tags, plus a gold path
  score, then nll = sum(logZ - gold) / 256.

The forward recurrence  alpha' = logsumexp_i(alpha_i + trans_ij) + emit_j
is run entirely in exp space:  with E = exp(trans), F_t = exp(emit_t - k),
  u_{t+1} = (E^T u_t) * F_{t+1}        (one matmul + one elementwise mul)
  logZ    = log(sum_j u_last[j] * exp(end_j)) + k*(S-1)
A fixed pre-scale k ~= log(T) + 1/2 keeps u in a tiny dynamic range
(empirically exp([-10, +6]) for N(0,1) emissions), so no per-step
normalization is needed and f32 is exact to ~1e-8 relative.

Sharding: data-parallel over props — 32 props per core on 8 cores; the
[66,66] exp(transitions) matrix is replicated. Each scan step is a single
[66,66]x[66,32] PE matmul plus a [66,32] vector multiply; the 127-step
serial chain is the kernel's critical path.

Host side does the (tiny) gathers: predicate-row gather from `score`,
gold path score, exp() pre-scaling, and the final scalar reduction.
"""

import os
import sys

import numpy as np

for _p in ("/opt/trn_rl_repo",):
    if os.path.isdir(_p) and _p not in sys.path:
        sys.path.insert(0, _p)

import concourse.bass as bass
import concourse.mybir as mybir
import concourse.tile as tile
from concourse import bacc
from concourse.bass_utils import run_bass_kernel_spmd

B, S, V, T = 32, 128, 8, 66
N_CORES = 8
BV = B * V
P = BV // N_CORES          # 32 props per core
NSTEP = S - 1              # 127 scan steps
KAPPA = float(np.float32(4.7))   # per-step pre-scale, added back at the end

# knobs (test.py may override before first kernel() call)
PROFILE = False
TRACE_TMPDIR = None
F_CHUNK_STEPS = 16         # emissions DMA chunking (steps per chunk)
LDW_REUSE = True           # skip per-step LDWEIGHTS via InstMatmult.ldweights=False
LAST_RESULTS = None        # BassKernelResults of the last run (for profiling)

_nc_cache = {}


def _build_bass():
    # Bacc (not plain Bass): its finalize() runs move_matmul_waits_to_ldweights
    # + generate_event_semaphores, which split multi-semaphore waits that the
    # TRN2 ISA can't encode on a single instruction.
    nc = bacc.Bacc()
    f32 = mybir.dt.float32
    f16 = mybir.dt.float16

    # The PE matmul runs fp16 at 1 cycle/row (fp32 needs a 2-pass split at
    # 4 cycles/row), and exact-fp16 inputs accumulate in fp32 PSUM, which
    # keeps the overall nll error ~1e-6 relative. E, u0 and a ones column
    # (for the final reduction) are packed into one fp16 tensor -> one DMA
    # -> one semaphore, since PE Matmult only supports a single sync wait.
    NCONST = T + P + 1
    c_in = nc.dram_tensor("consts", [T, NCONST], f16, kind="ExternalInput")
    f_in = nc.dram_tensor("f_exp", [T, NSTEP * P], f32, kind="ExternalInput")
    z_out = nc.dram_tensor("z_out", [1, P], f32, kind="ExternalOutput")

    with tile.TileContext(nc) as tc:
        with tc.tile_pool(name="const", bufs=1) as const, \
             tc.tile_pool(name="state", bufs=3) as state, \
             tc.tile_pool(name="ps", bufs=2, space="PSUM") as ps:
            c_sb = const.tile([T, NCONST], f16)
            nc.sync.dma_start(out=c_sb, in_=c_in[:, :])
            E_sb = c_sb[:, 0:T]
            u0_sb = c_sb[:, T:T + P]
            ones_sb = c_sb[:, T + P:T + P + 1]

            F_sb = const.tile([T, NSTEP * P], f32)
            for c0 in range(0, NSTEP, F_CHUNK_STEPS):
                c1 = min(NSTEP, c0 + F_CHUNK_STEPS)
                nc.sync.dma_start(
                    out=F_sb[:, c0 * P:c1 * P], in_=f_in[:, c0 * P:c1 * P]
                )

            if LDW_REUSE:
                # Load E into the PE array once; every scan matmul then skips
                # its LDWEIGHTS pass (ldweights=False). No other PE op touches
                # the stationary array until the final reduction matmul.
                nc.tensor.ldweights(E_sb)
            u_cur = u0_sb
            for t in range(NSTEP):
                v_ps = ps.tile([T, P], f32)
                mm = nc.tensor.matmul(v_ps, E_sb, u_cur, start=True, stop=True)
                if LDW_REUSE:
                    mm.ins.ldweights = False
                u_nxt = state.tile([T, P], f16)
                nc.vector.tensor_mul(u_nxt, v_ps, F_sb[:, t * P:(t + 1) * P])
                u_cur = u_nxt

            # exp(end) is folded into the last F block host-side, so the final
            # reduction is a plain column sum against an exact fp16 ones vector.
            zf_ps = ps.tile([1, P], f32)
            nc.tensor.matmul(zf_ps, ones_sb, u_cur, start=True, stop=True)
            z_sb = state.tile([1, P], f32)
            nc.scalar.activation(z_sb, zf_ps, mybir.ActivationFunctionType.Ln)
            nc.sync.dma_start(out=z_out[:, :], in_=z_sb)

    nc.finalize()
    return nc


def _get_nc():
    key = ("crf", T, P, NSTEP, F_CHUNK_STEPS, LDW_REUSE)
    if key not in _nc_cache:
        _nc_cache[key] = _build_bass()
    return _nc_cache[key]


def kernel(score, transitions, start_transitions, end_transitions,
           v_label, role_label):
    global LAST_RESULTS
    score = np.asarray(score, dtype=np.float32)
    transitions = np.asarray(transitions, dtype=np.float32)
    start_transitions = np.asarray(start_transitions, dtype=np.float32)
    end_transitions = np.asarray(end_transitions, dtype=np.float32)
    vl = np.asarray(v_label).astype(np.int64)
    rl = np.asarray(role_label).astype(np.int64)

    # gather predicate rows: emissions[b*V+v] = score[b, v_label[b,v]]  [BV,S,T]
    em = np.take_along_axis(score, vl[:, :, None, None], axis=1).reshape(BV, S, T)
    tags = rl.reshape(BV, S)

    # gold path score (host, f64)
    ar = np.arange(BV)
    emit_sc = em[ar[:, None], np.arange(S)[None, :], tags].astype(np.float64).sum(-1)
    tr64 = transitions.astype(np.float64)
    trans_sc = tr64[tags[:, :-1], tags[:, 1:]].sum(-1)
    gold = (start_transitions.astype(np.float64)[tags[:, 0]] + emit_sc
            + trans_sc + end_transitions.astype(np.float64)[tags[:, -1]])

    # device inputs
    E = np.exp(transitions)                                   # [T,T] f32
    u0 = np.exp(start_transitions[:, None] + em[:, 0, :].T)   # [T,BV] f32
    # F[j, t, p] = exp(em[p, t+1, j] - kappa); exp(end) folded into the last step
    Ft = np.exp(np.transpose(em[:, 1:, :], (2, 1, 0)) - np.float32(KAPPA))
    Ft[:, -1, :] *= np.exp(end_transitions)[:, None]

    nc = _get_nc()
    in_maps = []
    ones = np.ones((T, 1), np.float16)
    for m in range(N_CORES):
        sl = slice(m * P, (m + 1) * P)
        consts = np.concatenate(
            [E.astype(np.float16), u0[:, sl].astype(np.float16), ones], axis=1)
        in_maps.append({
            "consts": np.ascontiguousarray(consts),
            "f_exp": np.ascontiguousarray(Ft[:, :, sl]).reshape(T, NSTEP * P),
        })

    kwargs = {}
    if PROFILE:
        kwargs.update(trace=True, tmpdir=TRACE_TMPDIR)
    res = run_bass_kernel_spmd(nc, in_maps, list(range(N_CORES)), **kwargs)
    LAST_RESULTS = res

    z = np.concatenate([res.results[m]["z_out"][0] for m in range(N_CORES)])
    logz = z.astype(np.float64) + KAPPA * NSTEP
    nll = (logz - gold).sum() / BV
    return np.float32(nll)


# revision 15
# speedup vs baseline: 1.0092x; 1.0092x over previous
"""CRF loss (nn_CRFLoss) on 8 Trainium2 NeuronCores.

Strategy
--------
The reference computes, per proposition (B*V = 256 of them):
  logZ via a 128-step forward algorithm over T=66 tags, plus a gold path
  score, then nll = sum(logZ - gold) / 256.

The forward recurrence  alpha' = logsumexp_i(alpha_i + trans_ij) + emit_j
is run entirely in exp space:  with E = exp(trans), F_t = exp(emit_t - k),
  u_{t+1} = (E^T u_t) * F_{t+1}        (one matmul + one elementwise mul)
  logZ    = log(sum_j u_last[j] * exp(end_j)) + k*(S-1)
A fixed pre-scale k ~= log(T) + 1/2 keeps u in a tiny dynamic range
(empirically exp([-10, +6]) for N(0,1) emissions), so no per-step
normalization is needed and f32 is exact to ~1e-8 relative.

Sharding: data-parallel over props — 32 props per core on 8 cores; the
[66,66] exp(transitions) matrix is replicated. Each scan step is a single
[66,66]x[66,32] PE matmul plus a [66,32] vector multiply; the 127-step
serial chain is the kernel's critical path.

Host side does the (tiny) gathers: predicate-row gather from `score`,
gold path score, exp() pre-scaling, and the final scalar reduction.
"""

import os
import sys

import numpy as np

for _p in ("/opt/trn_rl_repo",):
    if os.path.isdir(_p) and _p not in sys.path:
        sys.path.insert(0, _p)

import concourse.bass as bass
import concourse.mybir as mybir
import concourse.tile as tile
from concourse import bacc
from concourse.bass_utils import run_bass_kernel_spmd

B, S, V, T = 32, 128, 8, 66
N_CORES = 8
BV = B * V
P = BV // N_CORES          # 32 props per core
NSTEP = S - 1              # 127 scan steps
KAPPA = float(np.float32(4.7))   # per-step pre-scale, added back at the end

# knobs (test.py may override before first kernel() call)
PROFILE = False
TRACE_TMPDIR = None
F_CHUNK_STEPS = 16         # emissions DMA chunking (steps per chunk)
LDW_REUSE = True           # skip per-step LDWEIGHTS via InstMatmult.ldweights=False
LAST_RESULTS = None        # BassKernelResults of the last run (for profiling)

_nc_cache = {}


def _build_bass():
    # Bacc (not plain Bass): its finalize() runs move_matmul_waits_to_ldweights
    # + generate_event_semaphores, which split multi-semaphore waits that the
    # TRN2 ISA can't encode on a single instruction.
    nc = bacc.Bacc()
    f32 = mybir.dt.float32
    f16 = mybir.dt.float16

    # The PE matmul runs fp16 at 1 cycle/row (fp32 needs a 2-pass split at
    # 4 cycles/row), and exact-fp16 inputs accumulate in fp32 PSUM, which
    # keeps the overall nll error ~1e-6 relative. E and u0 are packed into
    # one fp16 tensor -> one DMA -> one semaphore, since PE Matmult only
    # supports a single sync wait.
    NCONST = T + P
    c_in = nc.dram_tensor("consts", [T, NCONST], f16, kind="ExternalInput")
    f_in = nc.dram_tensor("f_exp", [T, NSTEP * P], f32, kind="ExternalInput")
    u_out = nc.dram_tensor("u_out", [T, P], f16, kind="ExternalOutput")

    with tile.TileContext(nc) as tc:
        with tc.tile_pool(name="const", bufs=1) as const, \
             tc.tile_pool(name="state", bufs=3) as state, \
             tc.tile_pool(name="ps", bufs=2, space="PSUM") as ps:
            c_sb = const.tile([T, NCONST], f16)
            nc.sync.dma_start(out=c_sb, in_=c_in[:, :])
            E_sb = c_sb[:, 0:T]
            u0_sb = c_sb[:, T:T + P]

            F_sb = const.tile([T, NSTEP * P], f32)
            for c0 in range(0, NSTEP, F_CHUNK_STEPS):
                c1 = min(NSTEP, c0 + F_CHUNK_STEPS)
                nc.sync.dma_start(
                    out=F_sb[:, c0 * P:c1 * P], in_=f_in[:, c0 * P:c1 * P]
                )

            if LDW_REUSE:
                # Load E into the PE array once; every scan matmul then skips
                # its LDWEIGHTS pass (ldweights=False). No other PE op touches
                # the stationary array until the final reduction matmul.
                nc.tensor.ldweights(E_sb)
            u_cur = u0_sb
            for t in range(NSTEP):
                v_ps = ps.tile([T, P], f32)
                mm = nc.tensor.matmul(v_ps, E_sb, u_cur, start=True, stop=True)
                if LDW_REUSE:
                    mm.ins.ldweights = False
                u_nxt = state.tile([T, P], f16)
                nc.vector.tensor_mul(u_nxt, v_ps, F_sb[:, t * P:(t + 1) * P])
                u_cur = u_nxt

            # exp(end) is folded into the last F block host-side; the final
            # column-sum + log runs on the host, so the device tail is just
            # one small DMA (no final matmul / Ln-table load on the tail).
            nc.sync.dma_start(out=u_out[:, :], in_=u_cur)

    nc.finalize()
    return nc


def _get_nc():
    key = ("crf", T, P, NSTEP, F_CHUNK_STEPS, LDW_REUSE)
    if key not in _nc_cache:
        _nc_cache[key] = _build_bass()
    return _nc_cache[key]


def kernel(score, transitions, start_transitions, end_transitions,
           v_label, role_label):
    global LAST_RESULTS
    score = np.asarray(score, dtype=np.float32)
    transitions = np.asarray(transitions, dtype=np.float32)
    start_transitions = np.asarray(start_transitions, dtype=np.float32)
    end_transitions = np.asarray(end_transitions, dtype=np.float32)
    vl = np.asarray(v_label).astype(np.int64)
    rl = np.asarray(role_label).astype(np.int64)

    # gather predicate rows: emissions[b*V+v] = score[b, v_label[b,v]]  [BV,S,T]
    em = np.take_along_axis(score, vl[:, :, None, None], axis=1).reshape(BV, S, T)
    tags = rl.reshape(BV, S)

    # gold path score (host, f64)
    ar = np.arange(BV)
    emit_sc = em[ar[:, None], np.arange(S)[None, :], tags].astype(np.float64).sum(-1)
    tr64 = transitions.astype(np.float64)
    trans_sc = tr64[tags[:, :-1], tags[:, 1:]].sum(-1)
    gold = (start_transitions.astype(np.float64)[tags[:, 0]] + emit_sc
            + trans_sc + end_transitions.astype(np.float64)[tags[:, -1]])

    # device inputs
    E = np.exp(transitions)                                   # [T,T] f32
    u0 = np.exp(start_transitions[:, None] + em[:, 0, :].T)   # [T,BV] f32
    # F[j, t, p] = exp(em[p, t+1, j] - kappa); exp(end) folded into the last step
    Ft = np.exp(np.transpose(em[:, 1:, :], (2, 1, 0)) - np.float32(KAPPA))
    Ft[:, -1, :] *= np.exp(end_transitions)[:, None]

    nc = _get_nc()
    in_maps = []
    for m in range(N_CORES):
        sl = slice(m * P, (m + 1) * P)
        consts = np.concatenate(
            [E.astype(np.float16), u0[:, sl].astype(np.float16)], axis=1)
        in_maps.append({
            "consts": np.ascontiguousarray(consts),
            "f_exp": np.ascontiguousarray(Ft[:, :, sl]).reshape(T, NSTEP * P),
        })

    kwargs = {}
    if PROFILE:
        kwargs.update(trace=True, tmpdir=TRACE_TMPDIR)
    res = run_bass_kernel_spmd(nc, in_maps, list(range(N_CORES)), **kwargs)
    LAST_RESULTS = res

    u_last = np.concatenate(
        [res.results[m]["u_out"] for m in range(N_CORES)], axis=1)  # [T, BV]
    logz = np.log(u_last.astype(np.float64).sum(0)) + KAPPA * NSTEP
    nll = (logz - gold).sum() / BV
    return np.float32(nll)


# revision 16
# speedup vs baseline: 1.6289x; 1.6141x over previous
"""CRF loss (nn_CRFLoss) on 8 Trainium2 NeuronCores.

Strategy
--------
The reference computes, per proposition (B*V = 256 of them):
  logZ via a 128-step forward algorithm over T=66 tags, plus a gold path
  score, then nll = sum(logZ - gold) / 256.

The forward recurrence  alpha' = logsumexp_i(alpha_i + trans_ij) + emit_j
is run entirely in exp space:  with E = exp(trans), F_t = exp(emit_t - k),
  u_{t+1} = (E^T u_t) * F_{t+1}        (one matmul + one elementwise mul)
  logZ    = log(sum_j u_last[j] * exp(end_j)) + k*(S-1)
A fixed pre-scale k ~= log(T) + 1/2 keeps u in a tiny dynamic range
(empirically exp([-10, +6]) for N(0,1) emissions), so no per-step
normalization is needed.

The serial scan latency is halved by splitting it into a FORWARD chain
(alpha, steps 1..64) and a BACKWARD chain (beta, steps 127..65) that meet
in the middle:  Z = sum_j alpha_64[j] * beta_64[j].  Both chains have the
same matmul+multiply step shape (backward uses E instead of E^T as the PE
stationary) and interleave on the Tensor/Vector engines, so the ~64-step
chain latency — not the 127 matmuls — bounds the wall clock.

Matmuls run in fp16 (1 cycle/row on the PE vs 4 for fp32) with fp32 PSUM
accumulation; overall nll error vs the f32 reference is ~1e-6 relative.

Sharding: data-parallel over props — 32 props per core on 8 cores; the
tiny [66,66] transition matrices are replicated. Host side does the cheap
gathers (predicate rows from `score`, gold path score), the exp()
pre-scaling, and the final log+reduction of the per-prop partials.
"""

import os
import sys

import numpy as np

for _p in ("/opt/trn_rl_repo",):
    if os.path.isdir(_p) and _p not in sys.path:
        sys.path.insert(0, _p)

import concourse.bass as bass
import concourse.mybir as mybir
import concourse.tile as tile
from concourse import bacc
from concourse.bass_utils import run_bass_kernel_spmd

B, S, V, T = 32, 128, 8, 66
N_CORES = 8
BV = B * V
P = BV // N_CORES          # 32 props per core
NSTEP = S - 1              # 127 transition steps total
MID = 64                   # forward chain covers steps 1..MID
NBWD_MM = NSTEP - MID      # 63 backward matmuls (steps 127..65)
NF_DEV = NSTEP - 1         # F blocks shipped to device (t=1..126)
KAPPA = float(np.float32(4.7))   # per-step pre-scale, added back at the end

# knobs (test.py may override before first kernel() call)
PROFILE = False
TRACE_TMPDIR = None
F_CHUNK_STEPS = 16         # emissions DMA chunking (steps per chunk)
LAST_RESULTS = None        # BassKernelResults of the last run (for profiling)

_nc_cache = {}


def _build_bass():
    # Bacc (not plain Bass): its finalize() runs move_matmul_waits_to_ldweights
    # + generate_event_semaphores, which split multi-semaphore waits that the
    # TRN2 ISA can't encode on a single instruction.
    nc = bacc.Bacc()
    f32 = mybir.dt.float32
    f16 = mybir.dt.float16

    # E, E^T and both chains' initial states packed into one fp16 tensor ->
    # one DMA -> one semaphore, since PE Matmult only supports a single
    # sync wait.
    NCONST = 2 * T + 2 * P
    c_in = nc.dram_tensor("consts", [T, NCONST], f16, kind="ExternalInput")
    f_in = nc.dram_tensor("f_exp", [T, NF_DEV * P], f32, kind="ExternalInput")
    prod_out = nc.dram_tensor("prod_out", [T, P], f32, kind="ExternalOutput")

    with tile.TileContext(nc) as tc:
        with tc.tile_pool(name="const", bufs=1) as const, \
             tc.tile_pool(name="state", bufs=3) as state, \
             tc.tile_pool(name="ps", bufs=2, space="PSUM") as ps:
            c_sb = const.tile([T, NCONST], f16)
            nc.sync.dma_start(out=c_sb, in_=c_in[:, :])
            E_sb = c_sb[:, 0:T]
            Et_sb = c_sb[:, T:2 * T]
            u0_sb = c_sb[:, 2 * T:2 * T + P]
            w0_sb = c_sb[:, 2 * T + P:2 * T + 2 * P]

            F_sb = const.tile([T, NF_DEV * P], f32)
            # issue chunks from both ends alternately: the forward chain
            # consumes F from t=1 up, the backward chain from t=126 down.
            nchunks = (NF_DEV + F_CHUNK_STEPS - 1) // F_CHUNK_STEPS
            order, lo, hi = [], 0, nchunks - 1
            while lo <= hi:
                order.append(lo)
                if hi != lo:
                    order.append(hi)
                lo, hi = lo + 1, hi - 1
            for c in order:
                c0, c1 = c * F_CHUNK_STEPS, min(NF_DEV, (c + 1) * F_CHUNK_STEPS)
                nc.sync.dma_start(
                    out=F_sb[:, c0 * P:c1 * P], in_=f_in[:, c0 * P:c1 * P]
                )

            u_cur, w_cur, b_ps = u0_sb, w0_sb, None
            for k in range(MID):
                # forward step t = k+1:  u' = (E^T u) * F_{k+1}
                v_ps = ps.tile([T, P], f32, tag="v")
                nc.tensor.matmul(v_ps, E_sb, u_cur, start=True, stop=True)
                u_nxt = state.tile([T, P], f16, tag="u")
                nc.vector.tensor_mul(u_nxt, v_ps, F_sb[:, k * P:(k + 1) * P])
                u_cur = u_nxt
                # backward step (k-th matmul: t = 127-k):  b = E w,
                # then w' = b * F_{126-k}
                if k < NBWD_MM:
                    b_ps = ps.tile([T, P], f32, tag="b")
                    nc.tensor.matmul(b_ps, Et_sb, w_cur, start=True, stop=True)
                    if k < NBWD_MM - 1:
                        w_nxt = state.tile([T, P], f16, tag="w")
                        nc.vector.tensor_mul(
                            w_nxt, b_ps, F_sb[:, (125 - k) * P:(126 - k) * P])
                        w_cur = w_nxt

            # meet in the middle: Z_p = sum_j u_64[j,p] * beta_64[j,p];
            # the column sum + log runs on the host.
            prod_sb = state.tile([T, P], f32, tag="prod")
            nc.vector.tensor_mul(prod_sb, b_ps, u_cur)
            nc.sync.dma_start(out=prod_out[:, :], in_=prod_sb)

    nc.finalize()
    return nc


def _get_nc():
    key = ("crf-fb", T, P, NSTEP, MID, F_CHUNK_STEPS)
    if key not in _nc_cache:
        _nc_cache[key] = _build_bass()
    return _nc_cache[key]


def kernel(score, transitions, start_transitions, end_transitions,
           v_label, role_label):
    global LAST_RESULTS
    score = np.asarray(score, dtype=np.float32)
    transitions = np.asarray(transitions, dtype=np.float32)
    start_transitions = np.asarray(start_transitions, dtype=np.float32)
    end_transitions = np.asarray(end_transitions, dtype=np.float32)
    vl = np.asarray(v_label).astype(np.int64)
    rl = np.asarray(role_label).astype(np.int64)

    # gather predicate rows: emissions[b*V+v] = score[b, v_label[b,v]]  [BV,S,T]
    em = np.take_along_axis(score, vl[:, :, None, None], axis=1).reshape(BV, S, T)
    tags = rl.reshape(BV, S)

    # gold path score (host, f64)
    ar = np.arange(BV)
    emit_sc = em[ar[:, None], np.arange(S)[None, :], tags].astype(np.float64).sum(-1)
    tr64 = transitions.astype(np.float64)
    trans_sc = tr64[tags[:, :-1], tags[:, 1:]].sum(-1)
    gold = (start_transitions.astype(np.float64)[tags[:, 0]] + emit_sc
            + trans_sc + end_transitions.astype(np.float64)[tags[:, -1]])

    # device inputs
    E = np.exp(transitions)                                   # [T,T] f32
    u0 = np.exp(start_transitions[:, None] + em[:, 0, :].T)   # [T,BV] f32
    # F[j, t, p] = exp(em[p, t+1, j] - kappa); exp(end) folded into the last
    # step, which seeds the backward chain (w_init = F_127 * 1).
    Ft = np.exp(np.transpose(em[:, 1:, :], (2, 1, 0)) - np.float32(KAPPA))
    Ft[:, -1, :] *= np.exp(end_transitions)[:, None]

    nc = _get_nc()
    in_maps = []
    E16 = E.astype(np.float16)
    Et16 = np.ascontiguousarray(E.T).astype(np.float16)
    for m in range(N_CORES):
        sl = slice(m * P, (m + 1) * P)
        consts = np.concatenate(
            [E16, Et16, u0[:, sl].astype(np.float16),
             Ft[:, -1, sl].astype(np.float16)], axis=1)
        in_maps.append({
            "consts": np.ascontiguousarray(consts),
            "f_exp": np.ascontiguousarray(Ft[:, :NF_DEV, sl]).reshape(T, NF_DEV * P),
        })

    kwargs = {}
    if PROFILE:
        kwargs.update(trace=True, tmpdir=TRACE_TMPDIR)
    res = run_bass_kernel_spmd(nc, in_maps, list(range(N_CORES)), **kwargs)
    LAST_RESULTS = res

    prod = np.concatenate(
        [res.results[m]["prod_out"] for m in range(N_CORES)], axis=1)  # [T, BV]
    logz = np.log(prod.astype(np.float64).sum(0)) + KAPPA * NSTEP
    nll = (logz - gold).sum() / BV
    return np.float32(nll)


# revision 20
# speedup vs baseline: 1.6632x; 1.0211x over previous
"""CRF loss (nn_CRFLoss) on 8 Trainium2 NeuronCores.

Strategy
--------
The reference computes, per proposition (B*V = 256 of them):
  logZ via a 128-step forward algorithm over T=66 tags, plus a gold path
  score, then nll = sum(logZ - gold) / 256.

The forward recurrence  alpha' = logsumexp_i(alpha_i + trans_ij) + emit_j
is run entirely in exp space:  with E = exp(trans), F_t = exp(emit_t - k),
  u_{t+1} = (E^T u_t) * F_{t+1}        (one matmul + one elementwise mul)
  logZ    = log(sum_j u_last[j] * exp(end_j)) + k*(S-1)
A fixed pre-scale k ~= log(T) + 1/2 keeps u in a tiny dynamic range
(empirically exp([-10, +6]) for N(0,1) emissions), so no per-step
normalization is needed.

The serial scan latency is halved by splitting it into a FORWARD chain
(alpha, steps 1..64) and a BACKWARD chain (beta, steps 127..65) that meet
in the middle:  Z = sum_j alpha_64[j] * beta_64[j].  Both chains have the
same matmul+multiply step shape (backward uses E instead of E^T as the PE
stationary) and interleave on the Tensor/Vector engines, so the ~64-step
chain latency — not the 127 matmuls — bounds the wall clock.

Matmuls run in fp16 (1 cycle/row on the PE vs 4 for fp32) with fp32 PSUM
accumulation; overall nll error vs the f32 reference is ~1e-6 relative.

Sharding: data-parallel over props — 32 props per core on 8 cores; the
tiny [66,66] transition matrices are replicated. Host side does the cheap
gathers (predicate rows from `score`, gold path score), the exp()
pre-scaling, and the final log+reduction of the per-prop partials.
"""

import os
import sys

import numpy as np

for _p in ("/opt/trn_rl_repo",):
    if os.path.isdir(_p) and _p not in sys.path:
        sys.path.insert(0, _p)

import concourse.bass as bass
import concourse.mybir as mybir
import concourse.tile as tile
from concourse import bacc
from concourse.bass_utils import run_bass_kernel_spmd

B, S, V, T = 32, 128, 8, 66
N_CORES = 8
BV = B * V
P = BV // N_CORES          # 32 props per core
NSTEP = S - 1              # 127 transition steps total
MID = 64                   # forward chain covers steps 1..MID
NBWD_MM = NSTEP - MID      # 63 backward matmuls (steps 127..65)
NF_DEV = NSTEP - 1         # F blocks shipped to device (t=1..126)
KAPPA = float(np.float32(4.7))   # per-step pre-scale, added back at the end

# knobs (test.py may override before first kernel() call)
PROFILE = False
TRACE_TMPDIR = None
F_CHUNK_STEPS = 16         # emissions DMA chunking (steps per chunk)
LAST_RESULTS = None        # BassKernelResults of the last run (for profiling)

_nc_cache = {}


def _build_bass():
    # Bacc (not plain Bass): its finalize() runs move_matmul_waits_to_ldweights
    # + generate_event_semaphores, which split multi-semaphore waits that the
    # TRN2 ISA can't encode on a single instruction.
    nc = bacc.Bacc()
    f32 = mybir.dt.float32
    f16 = mybir.dt.float16

    # E, E^T and both chains' initial states packed into one fp16 tensor ->
    # one DMA -> one semaphore, since PE Matmult only supports a single
    # sync wait.
    NCONST = 2 * T + 2 * P
    c_in = nc.dram_tensor("consts", [T, NCONST], f16, kind="ExternalInput")
    f_in = nc.dram_tensor("f_exp", [T, NF_DEV * P], f16, kind="ExternalInput")
    prod_out = nc.dram_tensor("prod_out", [T, P], f32, kind="ExternalOutput")

    with tile.TileContext(nc) as tc:
        with tc.tile_pool(name="const", bufs=1) as const, \
             tc.tile_pool(name="state", bufs=3) as state, \
             tc.tile_pool(name="ps", bufs=2, space="PSUM") as ps:
            c_sb = const.tile([T, NCONST], f16)
            nc.sync.dma_start(out=c_sb, in_=c_in[:, :])
            E_sb = c_sb[:, 0:T]
            Et_sb = c_sb[:, T:2 * T]
            u0_sb = c_sb[:, 2 * T:2 * T + P]
            w0_sb = c_sb[:, 2 * T + P:2 * T + 2 * P]

            F_sb = const.tile([T, NF_DEV * P], f16)
            # issue chunks from both ends alternately (the forward chain
            # consumes F from t=1 up, the backward chain from t=126 down),
            # with small head chunks so both chains can start ASAP.
            def _ranges(lo, hi, first_small):
                out, c = [], lo
                sizes = [first_small] if first_small else []
                while c < hi:
                    sz = sizes.pop(0) if sizes else F_CHUNK_STEPS
                    out.append((c, min(hi, c + sz)))
                    c = min(hi, c + sz)
                return out
            fwd_chunks = _ranges(0, MID, 4)
            bwd_chunks = [(NF_DEV - b, NF_DEV - a)
                          for (a, b) in _ranges(0, NF_DEV - MID, 4)]
            order, i = [], 0
            while i < max(len(fwd_chunks), len(bwd_chunks)):
                if i < len(fwd_chunks):
                    order.append(fwd_chunks[i])
                if i < len(bwd_chunks):
                    order.append(bwd_chunks[i])
                i += 1
            for c0, c1 in order:
                nc.sync.dma_start(
                    out=F_sb[:, c0 * P:c1 * P], in_=f_in[:, c0 * P:c1 * P]
                )

            u_cur, w_cur = u0_sb, w0_sb
            v_last = None
            for k in range(MID):
                # forward step t = k+1:  u' = (E^T u) * F_{k+1}
                v_ps = ps.tile([T, P], f32, tag="v")
                nc.tensor.matmul(v_ps, E_sb, u_cur, start=True, stop=True)
                v_last = v_ps
                if k < MID - 1:
                    u_nxt = state.tile([T, P], f16, tag="u")
                    nc.vector.tensor_mul(u_nxt, v_ps, F_sb[:, k * P:(k + 1) * P])
                    u_cur = u_nxt
                # backward step (k-th matmul: t = 127-k):  b = E w,
                # then w' = b * F_{126-k}.  At k=62 this applies F_64 — the
                # last forward step's emission — moved onto the backward
                # chain so the forward critical path ends at its matmul:
                # sum_j (v*F)*beta == sum_j v*(F*beta).
                if k < NBWD_MM:
                    b_ps = ps.tile([T, P], f32, tag="b")
                    nc.tensor.matmul(b_ps, Et_sb, w_cur, start=True, stop=True)
                    w_nxt = state.tile([T, P], f16, tag="w")
                    nc.vector.tensor_mul(
                        w_nxt, b_ps, F_sb[:, (125 - k) * P:(126 - k) * P])
                    w_cur = w_nxt

            # meet in the middle: Z_p = sum_j v_64[j,p] * (F*beta)_64[j,p];
            # the column sum + log runs on the host.
            prod_sb = state.tile([T, P], f32, tag="prod")
            nc.vector.tensor_mul(prod_sb, v_last, w_cur)
            nc.sync.dma_start(out=prod_out[:, :], in_=prod_sb)

    nc.finalize()
    return nc


def _get_nc():
    key = ("crf-fb", T, P, NSTEP, MID, F_CHUNK_STEPS)
    if key not in _nc_cache:
        _nc_cache[key] = _build_bass()
    return _nc_cache[key]


def kernel(score, transitions, start_transitions, end_transitions,
           v_label, role_label):
    global LAST_RESULTS
    score = np.asarray(score, dtype=np.float32)
    transitions = np.asarray(transitions, dtype=np.float32)
    start_transitions = np.asarray(start_transitions, dtype=np.float32)
    end_transitions = np.asarray(end_transitions, dtype=np.float32)
    vl = np.asarray(v_label).astype(np.int64)
    rl = np.asarray(role_label).astype(np.int64)

    # gather predicate rows: emissions[b*V+v] = score[b, v_label[b,v]]  [BV,S,T]
    em = np.take_along_axis(score, vl[:, :, None, None], axis=1).reshape(BV, S, T)
    tags = rl.reshape(BV, S)

    # gold path score (host, f64)
    ar = np.arange(BV)
    emit_sc = em[ar[:, None], np.arange(S)[None, :], tags].astype(np.float64).sum(-1)
    tr64 = transitions.astype(np.float64)
    trans_sc = tr64[tags[:, :-1], tags[:, 1:]].sum(-1)
    gold = (start_transitions.astype(np.float64)[tags[:, 0]] + emit_sc
            + trans_sc + end_transitions.astype(np.float64)[tags[:, -1]])

    # device inputs
    E = np.exp(transitions)                                   # [T,T] f32
    u0 = np.exp(start_transitions[:, None] + em[:, 0, :].T)   # [T,BV] f32
    # F[j, t, p] = exp(em[p, t+1, j] - kappa); exp(end) folded into the last
    # step, which seeds the backward chain (w_init = F_127 * 1).
    Ft = np.exp(np.transpose(em[:, 1:, :], (2, 1, 0)) - np.float32(KAPPA))
    Ft[:, -1, :] *= np.exp(end_transitions)[:, None]

    nc = _get_nc()
    in_maps = []
    E16 = E.astype(np.float16)
    Et16 = np.ascontiguousarray(E.T).astype(np.float16)
    for m in range(N_CORES):
        sl = slice(m * P, (m + 1) * P)
        consts = np.concatenate(
            [E16, Et16, u0[:, sl].astype(np.float16),
             Ft[:, -1, sl].astype(np.float16)], axis=1)
        in_maps.append({
            "consts": np.ascontiguousarray(consts),
            "f_exp": np.ascontiguousarray(
                Ft[:, :NF_DEV, sl].astype(np.float16)).reshape(T, NF_DEV * P),
        })

    kwargs = {}
    if PROFILE:
        kwargs.update(trace=True, tmpdir=TRACE_TMPDIR)
    res = run_bass_kernel_spmd(nc, in_maps, list(range(N_CORES)), **kwargs)
    LAST_RESULTS = res

    prod = np.concatenate(
        [res.results[m]["prod_out"] for m in range(N_CORES)], axis=1)  # [T, BV]
    logz = np.log(prod.astype(np.float64).sum(0)) + KAPPA * NSTEP
    nll = (logz - gold).sum() / BV
    return np.float32(nll)


# revision 21
# speedup vs baseline: 1.6658x; 1.0016x over previous
"""CRF loss (nn_CRFLoss) on 8 Trainium2 NeuronCores.

Strategy
--------
The reference computes, per proposition (B*V = 256 of them):
  logZ via a 128-step forward algorithm over T=66 tags, plus a gold path
  score, then nll = sum(logZ - gold) / 256.

The forward recurrence  alpha' = logsumexp_i(alpha_i + trans_ij) + emit_j
is run entirely in exp space:  with E = exp(trans), F_t = exp(emit_t - k),
  u_{t+1} = (E^T u_t) * F_{t+1}        (one matmul + one elementwise mul)
  logZ    = log(sum_j u_last[j] * exp(end_j)) + k*(S-1)
A fixed pre-scale k ~= log(T) + 1/2 keeps u in a tiny dynamic range
(empirically exp([-10, +6]) for N(0,1) emissions), so no per-step
normalization is needed.

The serial scan latency is halved by splitting it into a FORWARD chain
(alpha, steps 1..64) and a BACKWARD chain (beta, steps 127..65) that meet
in the middle:  Z = sum_j alpha_64[j] * beta_64[j].  Both chains have the
same matmul+multiply step shape (backward uses E instead of E^T as the PE
stationary) and interleave on the Tensor/Vector engines, so the ~64-step
chain latency — not the 127 matmuls — bounds the wall clock.

Matmuls run in fp16 (1 cycle/row on the PE vs 4 for fp32) with fp32 PSUM
accumulation; overall nll error vs the f32 reference is ~1e-6 relative.

Sharding: data-parallel over props — 32 props per core on 8 cores; the
tiny [66,66] transition matrices are replicated. Host side does the cheap
gathers (predicate rows from `score`, gold path score), the exp()
pre-scaling, and the final log+reduction of the per-prop partials.
"""

import os
import sys

import numpy as np

for _p in ("/opt/trn_rl_repo",):
    if os.path.isdir(_p) and _p not in sys.path:
        sys.path.insert(0, _p)

import concourse.bass as bass
import concourse.mybir as mybir
import concourse.tile as tile
from concourse import bacc
from concourse.bass_utils import run_bass_kernel_spmd

B, S, V, T = 32, 128, 8, 66
N_CORES = 8
BV = B * V
P = BV // N_CORES          # 32 props per core
NSTEP = S - 1              # 127 transition steps total
MID = 64                   # forward chain covers steps 1..MID
NBWD_MM = NSTEP - MID      # 63 backward matmuls (steps 127..65)
NF_DEV = NSTEP - 1         # F blocks shipped to device (t=1..126)
KAPPA = float(np.float32(4.7))   # per-step pre-scale, added back at the end

# knobs (test.py may override before first kernel() call)
PROFILE = False
TRACE_TMPDIR = None
F_CHUNK_STEPS = 16         # emissions DMA chunking (steps per chunk)
LAST_RESULTS = None        # BassKernelResults of the last run (for profiling)

_nc_cache = {}


def _build_bass():
    # Bacc (not plain Bass): its finalize() runs move_matmul_waits_to_ldweights
    # + generate_event_semaphores, which split multi-semaphore waits that the
    # TRN2 ISA can't encode on a single instruction.
    nc = bacc.Bacc()
    f32 = mybir.dt.float32
    f16 = mybir.dt.float16

    # E, E^T and both chains' initial states packed into one fp16 tensor ->
    # one DMA -> one semaphore, since PE Matmult only supports a single
    # sync wait.
    NCONST = 2 * T + 2 * P
    c_in = nc.dram_tensor("consts", [T, NCONST], f16, kind="ExternalInput")
    f_in = nc.dram_tensor("f_exp", [T, NF_DEV * P], f16, kind="ExternalInput")
    prod_out = nc.dram_tensor("prod_out", [T, P], f32, kind="ExternalOutput")

    with tile.TileContext(nc) as tc:
        with tc.tile_pool(name="const", bufs=1) as const, \
             tc.tile_pool(name="state", bufs=4) as state, \
             tc.tile_pool(name="ps", bufs=3, space="PSUM") as ps:
            c_sb = const.tile([T, NCONST], f16)
            nc.sync.dma_start(out=c_sb, in_=c_in[:, :])
            E_sb = c_sb[:, 0:T]
            Et_sb = c_sb[:, T:2 * T]
            u0_sb = c_sb[:, 2 * T:2 * T + P]
            w0_sb = c_sb[:, 2 * T + P:2 * T + 2 * P]

            F_sb = const.tile([T, NF_DEV * P], f16)
            # issue chunks from both ends alternately (the forward chain
            # consumes F from t=1 up, the backward chain from t=126 down),
            # with small head chunks so both chains can start ASAP.
            def _ranges(lo, hi, first_small):
                out, c = [], lo
                sizes = [first_small] if first_small else []
                while c < hi:
                    sz = sizes.pop(0) if sizes else F_CHUNK_STEPS
                    out.append((c, min(hi, c + sz)))
                    c = min(hi, c + sz)
                return out
            fwd_chunks = _ranges(0, MID, 4)
            bwd_chunks = [(NF_DEV - b, NF_DEV - a)
                          for (a, b) in _ranges(0, NF_DEV - MID, 4)]
            order, i = [], 0
            while i < max(len(fwd_chunks), len(bwd_chunks)):
                if i < len(fwd_chunks):
                    order.append(fwd_chunks[i])
                if i < len(bwd_chunks):
                    order.append(bwd_chunks[i])
                i += 1
            for c0, c1 in order:
                nc.sync.dma_start(
                    out=F_sb[:, c0 * P:c1 * P], in_=f_in[:, c0 * P:c1 * P]
                )

            u_cur, w_cur = u0_sb, w0_sb
            v_last = None
            for k in range(MID):
                # forward step t = k+1:  u' = (E^T u) * F_{k+1}
                v_ps = ps.tile([T, P], f32, tag="v")
                nc.tensor.matmul(v_ps, E_sb, u_cur, start=True, stop=True)
                v_last = v_ps
                if k < MID - 1:
                    u_nxt = state.tile([T, P], f16, tag="u")
                    nc.vector.tensor_mul(u_nxt, v_ps, F_sb[:, k * P:(k + 1) * P])
                    u_cur = u_nxt
                # backward step (k-th matmul: t = 127-k):  b = E w,
                # then w' = b * F_{126-k}.  At k=62 this applies F_64 — the
                # last forward step's emission — moved onto the backward
                # chain so the forward critical path ends at its matmul:
                # sum_j (v*F)*beta == sum_j v*(F*beta).
                if k < NBWD_MM:
                    b_ps = ps.tile([T, P], f32, tag="b")
                    nc.tensor.matmul(b_ps, Et_sb, w_cur, start=True, stop=True)
                    w_nxt = state.tile([T, P], f16, tag="w")
                    nc.vector.tensor_mul(
                        w_nxt, b_ps, F_sb[:, (125 - k) * P:(126 - k) * P])
                    w_cur = w_nxt

            # meet in the middle: Z_p = sum_j v_64[j,p] * (F*beta)_64[j,p];
            # the column sum + log runs on the host.
            prod_sb = state.tile([T, P], f32, tag="prod")
            nc.vector.tensor_mul(prod_sb, v_last, w_cur)
            nc.sync.dma_start(out=prod_out[:, :], in_=prod_sb)

    nc.finalize()
    return nc


def _get_nc():
    key = ("crf-fb", T, P, NSTEP, MID, F_CHUNK_STEPS)
    if key not in _nc_cache:
        _nc_cache[key] = _build_bass()
    return _nc_cache[key]


def kernel(score, transitions, start_transitions, end_transitions,
           v_label, role_label):
    global LAST_RESULTS
    score = np.asarray(score, dtype=np.float32)
    transitions = np.asarray(transitions, dtype=np.float32)
    start_transitions = np.asarray(start_transitions, dtype=np.float32)
    end_transitions = np.asarray(end_transitions, dtype=np.float32)
    vl = np.asarray(v_label).astype(np.int64)
    rl = np.asarray(role_label).astype(np.int64)

    # gather predicate rows: emissions[b*V+v] = score[b, v_label[b,v]]  [BV,S,T]
    em = np.take_along_axis(score, vl[:, :, None, None], axis=1).reshape(BV, S, T)
    tags = rl.reshape(BV, S)

    # gold path score (host, f64)
    ar = np.arange(BV)
    emit_sc = em[ar[:, None], np.arange(S)[None, :], tags].astype(np.float64).sum(-1)
    tr64 = transitions.astype(np.float64)
    trans_sc = tr64[tags[:, :-1], tags[:, 1:]].sum(-1)
    gold = (start_transitions.astype(np.float64)[tags[:, 0]] + emit_sc
            + trans_sc + end_transitions.astype(np.float64)[tags[:, -1]])

    # device inputs
    E = np.exp(transitions)                                   # [T,T] f32
    u0 = np.exp(start_transitions[:, None] + em[:, 0, :].T)   # [T,BV] f32
    # F[j, t, p] = exp(em[p, t+1, j] - kappa); exp(end) folded into the last
    # step, which seeds the backward chain (w_init = F_127 * 1).
    Ft = np.exp(np.transpose(em[:, 1:, :], (2, 1, 0)) - np.float32(KAPPA))
    Ft[:, -1, :] *= np.exp(end_transitions)[:, None]

    nc = _get_nc()
    in_maps = []
    E16 = E.astype(np.float16)
    Et16 = np.ascontiguousarray(E.T).astype(np.float16)
    for m in range(N_CORES):
        sl = slice(m * P, (m + 1) * P)
        consts = np.concatenate(
            [E16, Et16, u0[:, sl].astype(np.float16),
             Ft[:, -1, sl].astype(np.float16)], axis=1)
        in_maps.append({
            "consts": np.ascontiguousarray(consts),
            "f_exp": np.ascontiguousarray(
                Ft[:, :NF_DEV, sl].astype(np.float16)).reshape(T, NF_DEV * P),
        })

    kwargs = {}
    if PROFILE:
        kwargs.update(trace=True, tmpdir=TRACE_TMPDIR)
    res = run_bass_kernel_spmd(nc, in_maps, list(range(N_CORES)), **kwargs)
    LAST_RESULTS = res

    prod = np.concatenate(
        [res.results[m]["prod_out"] for m in range(N_CORES)], axis=1)  # [T, BV]
    logz = np.log(prod.astype(np.float64).sum(0)) + KAPPA * NSTEP
    nll = (logz - gold).sum() / BV
    return np.float32(nll)
